# revision 2
# baseline (speedup 1.0000x reference)
"""ContextWeaver: context[i, j] = relu(sum_{k,d} node[i,k,d] * edge[j,k,d]), diag zeroed.

Strategy (8 NeuronCores, SPMD):
  - Shard node rows 8-way (1024 rows/core); replicate edge^T per core with a
    per-core column rotation of c*1024 so the diagonal block lands at local
    columns [m*128, (m+1)*128) of every 128-row strip -- the instruction
    stream is identical on all cores and diagonal masking is fully static.
  - Everything in bf16 (inputs cast on host, output upcast on host): the
    correctness gate is rel_err < 2e-2 and bf16 end-to-end lands ~7e-3.
    This makes the matmuls 4x faster (fp32 is 4 cycles/row on the PE) and
    halves the output HBM traffic, which is the binding roofline
    (16 MiB/core / ~358 GB/s ~= 47 us).
  - Contraction dim is 64 (= K*D); pack two independent 64-row matmuls into
    the 128x128 PE array with tile_position row tiling: partitions 0-63
    compute local columns [0, 4096), partitions 64-127 compute [4096, 8192).
  - PSUM -> SBUF relu+bf16-cast entirely on VectorE (under the DMA roofline);
    ScalarE (ACT) and SP sequencers are kept compute-free so the two HWDGE
    rings drain output DMAs without queueing behind compute.
  - Per-strip [128, 8192] bf16 staging; 512 KiB output DMAs alternated
    across the two HWDGE rings; finer leading chunks on strip 0.
  - Host unshards by rotating each slab back, stacking, upcasting to f32.
"""

import os as _os

_os.environ.setdefault("JAX_PLATFORMS", "axon,cpu")

import numpy as np
import ml_dtypes

import concourse.bass as bass
import concourse.mybir as mybir
import concourse.tile as tile
from concourse import bacc
from concourse.bass_utils import run_bass_kernel_spmd

N = 8192          # nodes
F = 64            # contraction (K*D = 2*32)
NCORES = 8
SHARD = N // NCORES        # 1024 rows per core
HALF = N // 2              # 4096 local columns per PE row-group
MT = 128                   # output-row strip height
NT = 512                   # matmul moving free dim (one PSUM bank fp32)

F32 = mybir.dt.float32
BF16 = mybir.dt.bfloat16
NP_BF16 = ml_dtypes.bfloat16


def build_nc():
    nc = bacc.Bacc("TRN2", target_bir_lowering=False, debug=False)

    node2_d = nc.dram_tensor("node2", [64, SHARD], BF16, kind="ExternalInput")
    edge2_d = nc.dram_tensor("edge2", [128, HALF], BF16, kind="ExternalInput")
    mask_d = nc.dram_tensor("dmask", [128, MT], BF16, kind="ExternalInput")
    out_d = nc.dram_tensor("out", [SHARD, N], BF16, kind="ExternalOutput")

    n_strips = SHARD // MT           # 8
    n_chunks = HALF // NT            # 8 matmuls per half per strip

    with tile.TileContext(nc) as tc:
        with (
            tc.tile_pool(name="consts", bufs=1) as consts,
            tc.tile_pool(name="outp", bufs=3) as outp,
            tc.tile_pool(name="psp", bufs=8, space=bass.MemorySpace.PSUM) as psp,
        ):
            node_sb = consts.tile([128, SHARD], BF16)
            mask_sb = consts.tile([128, MT], BF16)
            edge_sb = consts.tile([128, HALF], BF16)

            # ordered so the bytes gating the first matmul land first
            nc.sync.dma_start(out=edge_sb[:, 0:NT], in_=edge2_d[:, 0:NT])
            nc.sync.dma_start(out=node_sb[0:64, :], in_=node2_d[:, :])
            nc.sync.dma_start(out=mask_sb[:], in_=mask_d[:, :])
            for j in range(1, n_chunks):
                nc.sync.dma_start(
                    out=edge_sb[:, j * NT:(j + 1) * NT],
                    in_=edge2_d[:, j * NT:(j + 1) * NT],
                )
            # duplicate nodeT into partitions 64-127 for the hi row-group
            nc.vector.tensor_copy(node_sb[64:128, :], node_sb[0:64, :])

            for m in range(n_strips):
                strip = outp.tile([128, N], BF16)
                lhs_lo = node_sb[0:64, m * MT:(m + 1) * MT]
                lhs_hi = node_sb[64:128, m * MT:(m + 1) * MT]
                nd = m // 4   # lo chunk whose columns contain the diag block
                # lo half first so the leading output DMA unblocks earliest
                for n in range(n_chunks):
                    ps = psp.tile([128, NT], F32)
                    nc.tensor.matmul(
                        ps[:], lhs_lo, edge_sb[0:64, n * NT:(n + 1) * NT],
                        start=True, stop=True, tile_position=(0, 0),
                    )
                    nc.vector.tensor_scalar_max(
                        strip[:, n * NT:(n + 1) * NT], ps[:], 0.0,
                    )
                    if n == nd:
                        # zero the diagonal block (always local cols
                        # [m*MT, (m+1)*MT), inside lo chunk m//4)
                        nc.vector.tensor_mul(
                            strip[:, m * MT:(m + 1) * MT],
                            strip[:, m * MT:(m + 1) * MT],
                            mask_sb[:],
                        )
                for n in range(n_chunks):
                    ps = psp.tile([128, NT], F32)
                    nc.tensor.matmul(
                        ps[:], lhs_hi, edge_sb[64:128, n * NT:(n + 1) * NT],
                        start=True, stop=True, tile_position=(64, 0),
                    )
                    nc.vector.tensor_scalar_max(
                        strip[:, HALF + n * NT:HALF + (n + 1) * NT], ps[:], 0.0,
                    )
                if m == 0:
                    # finer leading chunks shrink the ramp gap
                    bounds = [0, 512, 1024, 2048, 4096, 4608, 5120, 6144, 8192]
                else:
                    bounds = [0, 2048, 4096, 6144, 8192]
                for q, (lo, hi) in enumerate(zip(bounds[:-1], bounds[1:])):
                    eng = nc.scalar if q % 2 == 1 else nc.sync
                    eng.dma_start(
                        out=out_d[m * MT:(m + 1) * MT, lo:hi],
                        in_=strip[:, lo:hi],
                    )

    nc.compile()
    return nc


_NC = None


def _get_nc():
    global _NC
    if _NC is None:
        _NC = build_nc()
    return _NC


def make_in_maps(node_features: np.ndarray, edge_features: np.ndarray):
    node = np.ascontiguousarray(node_features, dtype=np.float32).reshape(N, F)
    edge = np.ascontiguousarray(edge_features, dtype=np.float32).reshape(N, F)
    edge_t = np.ascontiguousarray(edge.T).astype(NP_BF16)       # [64, 8192]
    mask = np.ones((128, MT), np.float32)
    np.fill_diagonal(mask, 0.0)
    mask = mask.astype(NP_BF16)

    in_maps = []
    for c in range(NCORES):
        node_t = node[c * SHARD:(c + 1) * SHARD].T.astype(NP_BF16)  # [64, 1024]
        node2 = np.ascontiguousarray(node_t)
        et = np.roll(edge_t, -c * SHARD, axis=1)   # local col j' = global (j'+c*1024)%N
        edge2 = np.ascontiguousarray(
            np.concatenate([et[:, :HALF], et[:, HALF:]], axis=0)
        )
        in_maps.append({"node2": node2, "edge2": edge2, "dmask": mask})
    return in_maps


def kernel(node_features: np.ndarray, edge_features: np.ndarray) -> np.ndarray:
    nc = _get_nc()
    in_maps = make_in_maps(node_features, edge_features)
    res = run_bass_kernel_spmd(nc, in_maps, core_ids=list(range(NCORES)))
    out = np.empty((N, N), np.float32)
    for c in range(NCORES):
        out[c * SHARD:(c + 1) * SHARD] = np.roll(
            res.results[c]["out"], c * SHARD, axis=1
        ).astype(np.float32)
    return out


# revision 4
# speedup vs baseline: 1.2657x; 1.2657x over previous
"""ContextWeaver: context[i, j] = relu(sum_{k,d} node[i,k,d] * edge[j,k,d]), diag zeroed.

Strategy (8 NeuronCores, SPMD):
  - Shard node rows 8-way (1024 rows/core); replicate edge^T per core with a
    per-core column rotation of c*1024 so the diagonal block lands at local
    columns [m*128, (m+1)*128) of every 128-row strip -- the instruction
    stream is identical on all cores and diagonal masking is fully static.
  - Everything in bf16 (inputs cast on host, output upcast on host): the
    correctness gate is rel_err < 2e-2 and bf16 end-to-end lands ~4e-3.
    bf16 matmuls stream 1 cycle/row (fp32 is 4) and the output HBM write --
    the binding roofline -- halves to 16 MiB/core (~47 us at ~360 GB/s).
  - Contraction dim is 64 (= K*D); pack two independent 64-row matmuls into
    the 128x128 PE array with tile_position row tiling: partitions 0-63
    compute local columns [0, 4096), partitions 64-127 compute [4096, 8192).
  - PSUM drain (relu + bf16 cast) is the second roofline: ~710 ns per
    [128,512] chunk per engine (PSUM source caps ACT/DVE at 1x), so it is
    split DVE=lo half (+ diag mask), ACT=hi half -> ~46 us each in parallel.
  - DMA rings: SP issues input loads + lo-half output pieces (waits on DVE),
    ACT issues hi-half pieces right after computing them -- neither engine's
    dma_start ever waits on the *other* compute engine, so issue never
    blocks compute.
  - Host unshards by rotating each slab back, stacking, upcasting to f32.
"""

import os as _os

_os.environ.setdefault("JAX_PLATFORMS", "axon,cpu")

import numpy as np
import ml_dtypes

import concourse.bass as bass
import concourse.mybir as mybir
import concourse.tile as tile
from concourse import bacc
from concourse.bass_utils import run_bass_kernel_spmd

N = 8192          # nodes
F = 64            # contraction (K*D = 2*32)
NCORES = 8
SHARD = N // NCORES        # 1024 rows per core
HALF = N // 2              # 4096 local columns per PE row-group
MT = 128                   # output-row strip height
NT = 512                   # matmul moving free dim (one PSUM bank fp32)

F32 = mybir.dt.float32
BF16 = mybir.dt.bfloat16
NP_BF16 = ml_dtypes.bfloat16


def build_nc():
    nc = bacc.Bacc("TRN2", target_bir_lowering=False, debug=False)

    node2_d = nc.dram_tensor("node2", [64, SHARD], BF16, kind="ExternalInput")
    edge2_d = nc.dram_tensor("edge2", [128, HALF], BF16, kind="ExternalInput")
    mask_d = nc.dram_tensor("dmask", [128, MT], BF16, kind="ExternalInput")
    out_d = nc.dram_tensor("out", [SHARD, N], BF16, kind="ExternalOutput")

    n_strips = SHARD // MT           # 8
    n_chunks = HALF // NT            # 8 matmuls per half per strip

    with tile.TileContext(nc) as tc:
        with (
            tc.tile_pool(name="consts", bufs=1) as consts,
            tc.tile_pool(name="outp", bufs=3) as outp,
            tc.tile_pool(name="psp", bufs=4, space=bass.MemorySpace.PSUM) as psp,
        ):
            node_sb = consts.tile([128, SHARD], BF16)
            mask_sb = consts.tile([128, MT], BF16)
            edge_sb = consts.tile([128, HALF], BF16)

            # ordered so the bytes gating the first matmul land first
            nc.sync.dma_start(out=edge_sb[:, 0:NT], in_=edge2_d[:, 0:NT])
            nc.sync.dma_start(out=node_sb[0:64, :], in_=node2_d[:, :])
            nc.sync.dma_start(out=mask_sb[:], in_=mask_d[:, :])
            for j in range(1, n_chunks):
                nc.sync.dma_start(
                    out=edge_sb[:, j * NT:(j + 1) * NT],
                    in_=edge2_d[:, j * NT:(j + 1) * NT],
                )
            # duplicate nodeT into partitions 64-127 for the hi row-group
            nc.vector.tensor_copy(node_sb[64:128, :], node_sb[0:64, :])

            for m in range(n_strips):
                strip = outp.tile([128, N], BF16)
                lhs_lo = node_sb[0:64, m * MT:(m + 1) * MT]
                lhs_hi = node_sb[64:128, m * MT:(m + 1) * MT]
                nd = m // 4   # lo chunk whose columns contain the diag block
                for n in range(n_chunks):
                    ps_a = psp.tile([128, NT], F32)
                    ps_b = psp.tile([128, NT], F32)
                    nc.tensor.matmul(
                        ps_a[:], lhs_lo, edge_sb[0:64, n * NT:(n + 1) * NT],
                        start=True, stop=True, tile_position=(0, 0),
                    )
                    nc.tensor.matmul(
                        ps_b[:], lhs_hi, edge_sb[64:128, n * NT:(n + 1) * NT],
                        start=True, stop=True, tile_position=(64, 0),
                    )
                    # DVE drains the lo half, ACT the hi half
                    nc.vector.tensor_scalar_max(
                        strip[:, n * NT:(n + 1) * NT], ps_a[:], 0.0,
                    )
                    nc.scalar.activation(
                        strip[:, HALF + n * NT:HALF + (n + 1) * NT], ps_b[:],
                        mybir.ActivationFunctionType.Relu,
                    )
                    if n == nd:
                        # zero the diagonal block (always local cols
                        # [m*MT, (m+1)*MT), inside lo chunk m//4)
                        nc.vector.tensor_mul(
                            strip[:, m * MT:(m + 1) * MT],
                            strip[:, m * MT:(m + 1) * MT],
                            mask_sb[:],
                        )
                # lo pieces on SP (waits on DVE; SP has nothing else to do),
                # hi pieces on ACT (its own data -- no cross-engine wait)
                if m == 0:
                    lo_bounds = [0, 512, 1024, 2048, 4096]
                    hi_bounds = [4096, 4608, 5120, 6144, 8192]
                else:
                    lo_bounds = [0, 2048, 4096]
                    hi_bounds = [4096, 6144, 8192]
                for lo, hi in zip(lo_bounds[:-1], lo_bounds[1:]):
                    nc.sync.dma_start(
                        out=out_d[m * MT:(m + 1) * MT, lo:hi],
                        in_=strip[:, lo:hi],
                    )
                for lo, hi in zip(hi_bounds[:-1], hi_bounds[1:]):
                    nc.scalar.dma_start(
                        out=out_d[m * MT:(m + 1) * MT, lo:hi],
                        in_=strip[:, lo:hi],
                    )

    nc.compile()
    return nc


_NC = None


def _get_nc():
    global _NC
    if _NC is None:
        _NC = build_nc()
    return _NC


def make_in_maps(node_features: np.ndarray, edge_features: np.ndarray):
    node = np.ascontiguousarray(node_features, dtype=np.float32).reshape(N, F)
    edge = np.ascontiguousarray(edge_features, dtype=np.float32).reshape(N, F)
    edge_t = np.ascontiguousarray(edge.T).astype(NP_BF16)       # [64, 8192]
    mask = np.ones((128, MT), np.float32)
    np.fill_diagonal(mask, 0.0)
    mask = mask.astype(NP_BF16)

    in_maps = []
    for c in range(NCORES):
        node_t = node[c * SHARD:(c + 1) * SHARD].T.astype(NP_BF16)  # [64, 1024]
        node2 = np.ascontiguousarray(node_t)
        et = np.roll(edge_t, -c * SHARD, axis=1)   # local col j' = global (j'+c*1024)%N
        edge2 = np.ascontiguousarray(
            np.concatenate([et[:, :HALF], et[:, HALF:]], axis=0)
        )
        in_maps.append({"node2": node2, "edge2": edge2, "dmask": mask})
    return in_maps


def kernel(node_features: np.ndarray, edge_features: np.ndarray) -> np.ndarray:
    nc = _get_nc()
    in_maps = make_in_maps(node_features, edge_features)
    res = run_bass_kernel_spmd(nc, in_maps, core_ids=list(range(NCORES)))
    out = np.empty((N, N), np.float32)
    for c in range(NCORES):
        out[c * SHARD:(c + 1) * SHARD] = np.roll(
            res.results[c]["out"], c * SHARD, axis=1
        ).astype(np.float32)
    return out


# revision 7
# speedup vs baseline: 1.3792x; 1.0897x over previous
"""ContextWeaver: context[i, j] = relu(sum_{k,d} node[i,k,d] * edge[j,k,d]), diag zeroed.

Strategy (8 NeuronCores, SPMD):
  - Shard node rows 8-way (1024 rows/core); replicate edge^T per core with a
    per-core column rotation of c*1024 so the diagonal block lands at local
    columns [m*128, (m+1)*128) of every 128-row strip -- the instruction
    stream is identical on all cores and diagonal masking is fully static.
  - Everything in bf16 (inputs cast on host, output upcast on host): the
    correctness gate is rel_err < 2e-2 and bf16 end-to-end lands ~4e-3.
    bf16 matmuls stream 1 cycle/row (fp32 is 4) and the output HBM write --
    the binding roofline -- halves to 16 MiB/core (~47 us at ~360 GB/s).
  - Contraction dim is 64 (= K*D); pack two independent 64-row matmuls into
    the 128x128 PE array with tile_position row tiling: partitions 0-63
    compute local columns [0, 4096), partitions 64-127 compute [4096, 8192).
  - PSUM drain (relu + bf16 cast) is the second roofline: ~710 ns per
    [128,512] chunk per engine (PSUM source caps ACT/DVE at 1x), so it is
    split DVE=lo half (+ diag mask), ACT=hi half -> ~46 us each in parallel.
  - ALL dma_start issues live on SP: a dma_start costs ~600 ns on the
    issuing sequencer, which would push ACT past the per-strip budget.
    SP is otherwise idle, so it eats every input load and output piece.
  - Host unshards by rotating each slab back, stacking, upcasting to f32.
"""

import os as _os

_os.environ.setdefault("JAX_PLATFORMS", "axon,cpu")

import numpy as np
import ml_dtypes

import concourse.bass as bass
import concourse.mybir as mybir
import concourse.tile as tile
from concourse import bacc
from concourse.bass_utils import run_bass_kernel_spmd

N = 8192          # nodes
F = 64            # contraction (K*D = 2*32)
NCORES = 8
SHARD = N // NCORES        # 1024 rows per core
HALF = N // 2              # 4096 local columns per PE row-group
MT = 128                   # output-row strip height
NT = 512                   # matmul moving free dim (one PSUM bank fp32)

F32 = mybir.dt.float32
BF16 = mybir.dt.bfloat16
NP_BF16 = ml_dtypes.bfloat16


def build_nc():
    nc = bacc.Bacc("TRN2", target_bir_lowering=False, debug=False)

    node2_d = nc.dram_tensor("node2", [64, SHARD], BF16, kind="ExternalInput")
    edge2_d = nc.dram_tensor("edge2", [128, HALF], BF16, kind="ExternalInput")
    mask_d = nc.dram_tensor("dmask", [128, MT], BF16, kind="ExternalInput")
    out_d = nc.dram_tensor("out", [SHARD, N], BF16, kind="ExternalOutput")

    n_strips = SHARD // MT           # 8
    n_chunks = HALF // NT            # 8 matmuls per half per strip

    with tile.TileContext(nc) as tc:
        with (
            tc.tile_pool(name="consts", bufs=1) as consts,
            tc.tile_pool(name="outp", bufs=4) as outp,
            tc.tile_pool(name="psp", bufs=4, space=bass.MemorySpace.PSUM) as psp,
        ):
            node_sb = consts.tile([128, SHARD], BF16)
            mask_sb = consts.tile([128, MT], BF16)
            edge_sb = consts.tile([128, HALF], BF16)

            # ordered so the bytes gating the first matmul land first
            nc.sync.dma_start(out=edge_sb[:, 0:NT], in_=edge2_d[:, 0:NT])
            nc.sync.dma_start(out=node_sb[0:64, :], in_=node2_d[:, :])
            nc.sync.dma_start(out=mask_sb[:], in_=mask_d[:, :])
            for j in range(1, n_chunks):
                nc.sync.dma_start(
                    out=edge_sb[:, j * NT:(j + 1) * NT],
                    in_=edge2_d[:, j * NT:(j + 1) * NT],
                )
            # duplicate nodeT into partitions 64-127 for the hi row-group
            nc.vector.tensor_copy(node_sb[64:128, :], node_sb[0:64, :])

            for m in range(n_strips):
                strip = outp.tile([128, N], BF16)
                lhs_lo = node_sb[0:64, m * MT:(m + 1) * MT]
                lhs_hi = node_sb[64:128, m * MT:(m + 1) * MT]
                nd = m // 4   # lo chunk whose columns contain the diag block
                for n in range(n_chunks):
                    ps_a = psp.tile([128, NT], F32)
                    ps_b = psp.tile([128, NT], F32)
                    nc.tensor.matmul(
                        ps_a[:], lhs_lo, edge_sb[0:64, n * NT:(n + 1) * NT],
                        start=True, stop=True, tile_position=(0, 0),
                    )
                    nc.tensor.matmul(
                        ps_b[:], lhs_hi, edge_sb[64:128, n * NT:(n + 1) * NT],
                        start=True, stop=True, tile_position=(64, 0),
                    )
                    # DVE drains the lo half, ACT the hi half
                    nc.vector.tensor_scalar_max(
                        strip[:, n * NT:(n + 1) * NT], ps_a[:], 0.0,
                    )
                    nc.scalar.activation(
                        strip[:, HALF + n * NT:HALF + (n + 1) * NT], ps_b[:],
                        mybir.ActivationFunctionType.Relu,
                    )
                    if n == nd:
                        # zero the diagonal block (always local cols
                        # [m*MT, (m+1)*MT), inside lo chunk m//4)
                        nc.vector.tensor_mul(
                            strip[:, m * MT:(m + 1) * MT],
                            strip[:, m * MT:(m + 1) * MT],
                            mask_sb[:],
                        )
                # all output pieces on SP, emitted in readiness order
                # (lo_0-3+mask, hi_0-3, lo_4-7, hi_4-7)
                if m == 0:
                    pieces = [(0, 1024), (4096, 5120), (1024, 2048),
                              (5120, 6144), (2048, 4096), (6144, 8192)]
                else:
                    pieces = [(0, 2048), (4096, 6144), (2048, 4096),
                              (6144, 8192)]
                for lo, hi in pieces:
                    nc.sync.dma_start(
                        out=out_d[m * MT:(m + 1) * MT, lo:hi],
                        in_=strip[:, lo:hi],
                    )

    nc.compile()
    return nc


_NC = None


def _get_nc():
    global _NC
    if _NC is None:
        _NC = build_nc()
    return _NC


def make_in_maps(node_features: np.ndarray, edge_features: np.ndarray):
    node = np.ascontiguousarray(node_features, dtype=np.float32).reshape(N, F)
    edge = np.ascontiguousarray(edge_features, dtype=np.float32).reshape(N, F)
    edge_t = np.ascontiguousarray(edge.T).astype(NP_BF16)       # [64, 8192]
    mask = np.ones((128, MT), np.float32)
    np.fill_diagonal(mask, 0.0)
    mask = mask.astype(NP_BF16)

    in_maps = []
    for c in range(NCORES):
        node_t = node[c * SHARD:(c + 1) * SHARD].T.astype(NP_BF16)  # [64, 1024]
        node2 = np.ascontiguousarray(node_t)
        et = np.roll(edge_t, -c * SHARD, axis=1)   # local col j' = global (j'+c*1024)%N
        edge2 = np.ascontiguousarray(
            np.concatenate([et[:, :HALF], et[:, HALF:]], axis=0)
        )
        in_maps.append({"node2": node2, "edge2": edge2, "dmask": mask})
    return in_maps


def kernel(node_features: np.ndarray, edge_features: np.ndarray) -> np.ndarray:
    nc = _get_nc()
    in_maps = make_in_maps(node_features, edge_features)
    res = run_bass_kernel_spmd(nc, in_maps, core_ids=list(range(NCORES)))
    out = np.empty((N, N), np.float32)
    for c in range(NCORES):
        out[c * SHARD:(c + 1) * SHARD] = np.roll(
            res.results[c]["out"], c * SHARD, axis=1
        ).astype(np.float32)
    return out
